# revision 14
# baseline (speedup 1.0000x reference)
"""Trainium2 Bass kernel for the VAE-style loss function.

Computes, from full inputs
    x, x_out: [256, 3, 128, 128] f32
    y:        [256, 7]  f32 (integer labels 0..9 with NaN = unlabeled)
    mu:       [256, 32] f32
    disc_pos: [10]      f32
the three scalars (recon, kld, recon + kld) exactly as the reference:
    recon   = |x - x_out|.sum(axis=(1,2,3)).mean()
    kld_d   = where(isnan(y_d), min_p (mu_d - pos_p)^2, (mu_d - pos[y_d])^2).mean(0).sum()
    kld_l   = where(isnan(y_l), relu(|mu_l| - 10)^2, (mu_l - y_l)^2).sum(1).mean()
    kld     = kld_d + kld_l

Strategy: pure data parallel over the batch dim across 8 NeuronCores.
Each core reduces its 32-sample slice to per-partition partial sums in
accR [128, NCOL]; the host sums partitions and cores and divides by 256.

Performance structure:
  - x/x_out staged entirely as fp8-e4m3 (quarters HBM bytes vs f32;
    the |.| sum over 1.5M elements averages out quantization noise,
    total rel err ~7e-4 vs the 2e-2 gate).
  - the subtraction runs on the Tensor engine as DoubleRow fp8
    identity matmuls: W = [I | -I] over the two k-tiles of each
    [128, 2, ch] chunk tile -> psum = x - x_out, exact.
  - abs+sum of the psum diffs is split per 2048-col chunk between ACT
    (Abs with accumulator) and DVE (tensor_reduce abs), 1024-col spans
    each, so both consumer ops are shorter than the PE fill and the
    pipeline stays PE-paced with no consumer stalls.
  - uniform 2048-col chunks: completion semaphores arrive every
    ~1.4us so the Tensor engine never waits on supply.
  - KLD vectorized over all dims, first in the DVE stream so it hides
    in the window before the first psum span lands.
"""

import numpy as np
import ml_dtypes

import concourse.bass as bass
import concourse.mybir as mybir
import concourse.bacc as bacc
import concourse.tile as tile


F32 = mybir.dt.float32
BF16 = mybir.dt.bfloat16
FP8 = mybir.dt.float8e4
NP_FP8 = ml_dtypes.float8_e4m3
ALU = mybir.AluOpType
AXIS = mybir.AxisListType
ACTF = mybir.ActivationFunctionType
PERF = mybir.MatmulPerfMode

N_CORES = 8
B = 256
BL = B // N_CORES          # 32 samples per core
P = 128                    # SBUF partitions
TOT = BL * 3 * 128 * 128   # 1572864 elements per big tensor per core
FREE = TOT // P            # 12288 elements per partition
CHUNKS = [2048, 2048, 2048, 2048, 2048, 2048]
assert sum(CHUNKS) == FREE
NCHUNK = len(CHUNKS)
CUM = [0]
for _c in CHUNKS:
    CUM.append(CUM[-1] + _c)

# Consumer plan: per chunk, a list of (engine, span) covering its cols.
# "act": ACT Abs+accum from a psum A-tile (<=1536); "dve": DVE
# tensor_reduce(abs) from a psum D-tile (<=512).
PLAN = [[("act", 1024), ("dve", 1024)] for _ in range(5)] + [
    [("act", 512), ("dve", 512), ("act", 512), ("dve", 512)]
]
N_ACT = sum(1 for p in PLAN for e, _ in p if e == "act")
N_DVE = sum(1 for p in PLAN for e, _ in p if e == "dve")
ND = 3                     # discrete dims
NL = 4                     # linear dims
NPOS = 10                  # codebook positions

# accR columns: [ACT cols | DVE cols | KLD]
COL_KLD = N_ACT + N_DVE
NCOL = COL_KLD + 1

# smalls packing, [BL, SM_W] f32 (iota/y broadcasts for the discrete and
# linear parts are packed ADJACENT so one is_equal covers both):
SM_MU3 = 0
SM_POS3 = 30
SM_IOTACAT = 60
SM_YBCAT = 130
SM_YCAT = 200
SM_MUL = 207
SM_W = 211


def build_module():
    nc = bacc.Bacc(
        "TRN2", target_bir_lowering=False, debug=False, num_devices=N_CORES
    )
    # x and x_out packed host-side per (chunk, partition) so that each
    # partition's chunk segment [x-cols || x_out-cols] is one contiguous
    # DRAM run -> one large DMA descriptor per partition per chunk.
    xc8 = nc.dram_tensor("xc8", [2 * P * FREE], FP8, kind="ExternalInput")
    wid = nc.dram_tensor("wid", [P, 2, P], FP8, kind="ExternalInput")
    sm = nc.dram_tensor("smalls", [BL, SM_W], F32, kind="ExternalInput")
    out = nc.dram_tensor("out", [P, NCOL], F32, kind="ExternalOutput")

    with tile.TileContext(nc) as tc:
        with (
            tc.tile_pool(name="big", bufs=1) as bp,
            tc.tile_pool(name="acc", bufs=1) as cp,
            tc.tile_pool(name="small", bufs=1) as sp,
            tc.tile_pool(name="work", bufs=1) as wp,
            tc.tile_pool(name="psum", bufs=1, space="PSUM") as pp,
        ):
            # identity weights FIRST on the Sync ring (every matmul gates
            # on them); smalls on the Scalar ring drain concurrently.
            wid_t = sp.tile([P, 2, P], FP8)
            nc.sync.dma_start(out=wid_t[:], in_=wid.ap())
            sm_t = sp.tile([BL, SM_W], F32)
            nc.scalar.dma_start(out=sm_t[:], in_=sm.ap())

            xts = []
            for i, ch in enumerate(CHUNKS):
                xt = bp.tile([P, 2, ch], FP8, tag=f"xt{i}")
                base = 2 * P * CUM[i]
                src = xc8.ap()[base : base + 2 * P * ch]
                src = src.rearrange("(p h n) -> p h n", p=P, h=2)
                nc.sync.dma_start(out=xt[:], in_=src)
                xts.append(xt)

            # ---- early setup (hides in the preamble window) ----
            ones_t = cp.tile([P, 1], F32)
            nc.vector.memset(ones_t[:], 1.0)
            accR = cp.tile([P, NCOL], F32)
            nc.vector.memset(accR[:], 0.0)
            # warm up the ACT table so the load is off the critical path
            warm = cp.tile([1, 1], F32)
            nc.vector.memset(warm[:], 0.0)
            nc.scalar.activation(warm[:], warm[:], ACTF.Abs)

            # psum tiles: 2x ACT spans + 2x DVE spans (2 banks each)
            # = exactly 8 banks.
            pa = [pp.tile([P, 1024], F32, name=f"pa{i}") for i in range(2)]
            pd = [pp.tile([P, 1024], F32, name=f"pd{i}") for i in range(2)]
            # ACT output dummies (bf16)
            ad = [bp.tile([P, 1024], BF16, name=f"ad{i}") for i in range(2)]

            # ---- KLD on the 32-sample rows, vectorized over dims ----
            mu3 = sm_t[:, SM_MU3 : SM_MU3 + 30]
            pos3 = sm_t[:, SM_POS3 : SM_POS3 + 30]
            iotacat = sm_t[:, SM_IOTACAT : SM_IOTACAT + 70]
            iota40 = sm_t[:, SM_IOTACAT + 30 : SM_IOTACAT + 70]
            ybcat = sm_t[:, SM_YBCAT : SM_YBCAT + 70]
            ycat = sm_t[:, SM_YCAT : SM_YCAT + ND + NL]
            mul = sm_t[:, SM_MUL : SM_MUL + NL]

            sel7 = wp.tile([BL, ND + NL], F32)

            ohcat = wp.tile([BL, 70], F32)
            nc.vector.tensor_tensor(ohcat[:], iotacat, ybcat, ALU.is_equal)
            oh = ohcat[:, 0:30]
            oh4 = ohcat[:, 30:70]
            eqcat = wp.tile([BL, ND + NL], F32)
            nc.vector.tensor_tensor(eqcat[:], ycat, ycat, ALU.is_equal)
            eqd = eqcat[:, 0:ND]
            eql = eqcat[:, ND:]

            dist = wp.tile([BL, 30], F32)
            nc.vector.tensor_sub(dist[:], mu3, pos3)
            nc.vector.tensor_mul(dist[:], dist[:], dist[:])
            nc.vector.tensor_mul(oh, oh, dist[:])
            unl = wp.tile([BL, ND], F32)
            nc.vector.tensor_reduce(
                unl[:], dist[:].rearrange("p (d k) -> p d k", k=NPOS),
                AXIS.X, ALU.min,
            )
            lab = wp.tile([BL, ND], F32)
            nc.vector.tensor_reduce(
                lab[:], oh.rearrange("p (d k) -> p d k", k=NPOS),
                AXIS.X, ALU.add,
            )
            nc.vector.tensor_sub(lab[:], lab[:], unl[:])
            nc.vector.tensor_mul(lab[:], lab[:], eqd)
            nc.vector.tensor_add(sel7[:, 0:ND], lab[:], unl[:])

            nc.vector.tensor_mul(oh4, oh4, iota40)
            ysafe = wp.tile([BL, NL], F32)
            nc.vector.tensor_reduce(
                ysafe[:], oh4.rearrange("p (d k) -> p d k", k=NPOS),
                AXIS.X, ALU.add,
            )
            labl = wp.tile([BL, NL], F32)
            nc.vector.tensor_sub(labl[:], mul, ysafe[:])
            nc.vector.tensor_mul(labl[:], labl[:], labl[:])
            nm = wp.tile([BL, NL], F32)
            nc.vector.tensor_scalar(nm[:], mul, -1.0, None, ALU.mult)
            nc.vector.tensor_max(nm[:], mul, nm[:])
            nc.vector.tensor_scalar(nm[:], nm[:], -10.0, 0.0, ALU.add, ALU.max)
            nc.vector.tensor_mul(nm[:], nm[:], nm[:])
            nc.vector.tensor_sub(labl[:], labl[:], nm[:])
            nc.vector.tensor_mul(labl[:], labl[:], eql)
            nc.vector.tensor_add(sel7[:, ND:], labl[:], nm[:])

            nc.vector.tensor_reduce(
                accR[0:BL, COL_KLD : COL_KLD + 1], sel7[:], AXIS.X, ALU.add
            )

            # ---- main loop: PE DoubleRow identity-sub into psum spans,
            # ACT/DVE abs+sum consumers ----
            col_act = 0
            col_dve = N_ACT
            ia = 0
            idv = 0

            for i, spans in enumerate(PLAN):
                xt = xts[i]
                off = 0
                for eng, width in spans:
                    if eng == "act":
                        pt = pa[ia % 2]
                        ia += 1
                    else:
                        pt = pd[idv % 2]
                        idv += 1
                    for j in range(0, width, 512):
                        w = min(512, width - j)
                        nc.tensor.matmul(
                            pt[:, j : j + w],
                            wid_t[:],
                            xt[:, :, off + j : off + j + w],
                            start=True, stop=True,
                            perf_mode=PERF.DoubleRow,
                        )
                    if eng == "act":
                        nc.scalar.activation(
                            ad[ia % 2][:, 0:width], pt[:, 0:width], ACTF.Abs,
                            accum_out=accR[:, col_act : col_act + 1],
                        )
                        col_act += 1
                    else:
                        nc.vector.tensor_reduce(
                            accR[:, col_dve : col_dve + 1], pt[:, 0:width],
                            AXIS.X, ALU.add,
                            apply_absolute_value=True,
                        )
                        col_dve += 1
                    off += width

            nc.sync.dma_start(out=out.ap(), in_=accR[:])

    nc.compile()
    return nc


_NC_CACHE = None


def _get_module():
    global _NC_CACHE
    if _NC_CACHE is None:
        _NC_CACHE = build_module()
    return _NC_CACHE


def make_in_maps(x, x_out, y, mu, disc_pos):
    x = np.asarray(x, dtype=np.float32)
    x_out = np.asarray(x_out, dtype=np.float32)
    y = np.asarray(y, dtype=np.float32)
    mu = np.asarray(mu, dtype=np.float32)
    disc_pos = np.asarray(disc_pos, dtype=np.float32)

    ident = np.eye(P, dtype=np.float32)
    wid = np.stack([ident, -ident], axis=1).astype(NP_FP8)

    iota = np.arange(NPOS, dtype=np.float32)
    in_maps = []
    for i in range(N_CORES):
        s = slice(i * BL, (i + 1) * BL)
        xv = x[s].reshape(P, FREE).astype(NP_FP8)
        yv = x_out[s].reshape(P, FREE).astype(NP_FP8)
        xc8 = np.empty(2 * P * FREE, dtype=NP_FP8)
        p8 = 0
        for k, ch in enumerate(CHUNKS):
            blk = np.stack(
                [xv[:, CUM[k]:CUM[k + 1]], yv[:, CUM[k]:CUM[k + 1]]],
                axis=1,
            )
            n = 2 * P * ch
            xc8[p8:p8 + n] = blk.reshape(-1)
            p8 += n

        mu_s, y_s = mu[s], y[s]
        sm = np.empty((BL, SM_W), dtype=np.float32)
        sm[:, SM_MU3:SM_MU3 + 30] = np.repeat(mu_s[:, :ND], NPOS, axis=1)
        sm[:, SM_POS3:SM_POS3 + 30] = np.tile(disc_pos, ND)
        sm[:, SM_IOTACAT:SM_IOTACAT + 70] = np.tile(iota, ND + NL)
        sm[:, SM_YBCAT:SM_YBCAT + 70] = np.repeat(y_s, NPOS, axis=1)
        sm[:, SM_YCAT:SM_YCAT + ND + NL] = y_s
        sm[:, SM_MUL:SM_MUL + NL] = mu_s[:, ND:ND + NL]

        in_maps.append({"xc8": xc8, "wid": wid, "smalls": sm})
    return in_maps


def combine_partials(partials):
    """partials: [8, P, NCOL] per-core accR -> full (3,) output."""
    p = np.asarray(partials, dtype=np.float64).reshape(N_CORES, P, NCOL)
    s = p.sum(axis=(0, 1)) / B
    recon = s[:COL_KLD].sum()
    kld = s[COL_KLD]
    return np.array([recon, kld, recon + kld], dtype=np.float32)


def run_spmd(x, x_out, y, mu, disc_pos, trace=False, **kw):
    from concourse.bass_utils import run_bass_kernel_spmd

    nc = _get_module()
    in_maps = make_in_maps(x, x_out, y, mu, disc_pos)
    r = run_bass_kernel_spmd(nc, in_maps, list(range(N_CORES)), trace=trace, **kw)
    partials = [r.results[i]["out"] for i in range(N_CORES)]
    return combine_partials(partials), r


def kernel(x, x_out, y, mu, disc_pos):
    out, _ = run_spmd(x, x_out, y, mu, disc_pos)
    return out


if __name__ == "__main__":
    nc = build_module()
    print("module built ok")


# revision 16
# speedup vs baseline: 1.0088x; 1.0088x over previous
"""Trainium2 Bass kernel for the VAE-style loss function.

Computes, from full inputs
    x, x_out: [256, 3, 128, 128] f32
    y:        [256, 7]  f32 (integer labels 0..9 with NaN = unlabeled)
    mu:       [256, 32] f32
    disc_pos: [10]      f32
the three scalars (recon, kld, recon + kld) exactly as the reference:
    recon   = |x - x_out|.sum(axis=(1,2,3)).mean()
    kld_d   = where(isnan(y_d), min_p (mu_d - pos_p)^2, (mu_d - pos[y_d])^2).mean(0).sum()
    kld_l   = where(isnan(y_l), relu(|mu_l| - 10)^2, (mu_l - y_l)^2).sum(1).mean()
    kld     = kld_d + kld_l

Strategy: pure data parallel over the batch dim across 8 NeuronCores.
Each core reduces its 32-sample slice to per-partition partial sums in
accR [128, NCOL]; the host sums partitions and cores and divides by 256.

Performance structure:
  - x/x_out staged entirely as fp8-e4m3 (quarters HBM bytes vs f32;
    the |.| sum over 1.5M elements averages out quantization noise,
    total rel err ~7e-4 vs the 2e-2 gate).
  - the subtraction runs on the Tensor engine as DoubleRow fp8
    identity matmuls: W = [I | -I] over the two k-tiles of each
    [128, 2, ch] chunk tile -> psum = x - x_out, exact.
  - abs+sum of the psum diffs is split per 2048-col chunk between ACT
    (Abs with accumulator) and DVE (tensor_reduce abs), 1024-col spans
    each, so both consumer ops are shorter than the PE fill and the
    pipeline stays PE-paced with no consumer stalls.
  - uniform 2048-col chunks: completion semaphores arrive every
    ~1.4us so the Tensor engine never waits on supply.
  - KLD vectorized over all dims, first in the DVE stream so it hides
    in the window before the first psum span lands.
"""

import numpy as np
import ml_dtypes

import concourse.bass as bass
import concourse.mybir as mybir
import concourse.bacc as bacc
import concourse.tile as tile


F32 = mybir.dt.float32
BF16 = mybir.dt.bfloat16
FP8 = mybir.dt.float8e4
NP_FP8 = ml_dtypes.float8_e4m3
ALU = mybir.AluOpType
AXIS = mybir.AxisListType
ACTF = mybir.ActivationFunctionType
PERF = mybir.MatmulPerfMode

N_CORES = 8
B = 256
BL = B // N_CORES          # 32 samples per core
P = 128                    # SBUF partitions
TOT = BL * 3 * 128 * 128   # 1572864 elements per big tensor per core
FREE = TOT // P            # 12288 elements per partition
CHUNKS = [1024, 1024, 2048, 2048, 2048, 2048, 2048]
assert sum(CHUNKS) == FREE
NCHUNK = len(CHUNKS)
CUM = [0]
for _c in CHUNKS:
    CUM.append(CUM[-1] + _c)

# Consumer plan: per chunk, a list of (engine, span) covering its cols.
# "act": ACT Abs+accum from a psum A-tile (<=1536); "dve": DVE
# tensor_reduce(abs) from a psum D-tile (<=512).
PLAN = [[("act", 1024)], [("dve", 1024)]] + [
    [("act", 1024), ("dve", 1024)] for _ in range(5)
]
N_ACT = sum(1 for p in PLAN for e, _ in p if e == "act")
N_DVE = sum(1 for p in PLAN for e, _ in p if e == "dve")
ND = 3                     # discrete dims
NL = 4                     # linear dims
NPOS = 10                  # codebook positions

# accR columns: [ACT cols | DVE cols | KLD]
COL_KLD = N_ACT + N_DVE
NCOL = COL_KLD + 1

# smalls packing, [BL, SM_W] f32 (iota/y broadcasts for the discrete and
# linear parts are packed ADJACENT so one is_equal covers both):
SM_MU3 = 0
SM_POS3 = 30
SM_IOTACAT = 60
SM_YBCAT = 130
SM_YCAT = 200
SM_MUL = 207
SM_W = 211


def build_module():
    nc = bacc.Bacc(
        "TRN2", target_bir_lowering=False, debug=False, num_devices=N_CORES
    )
    # x and x_out packed host-side per (chunk, partition) so that each
    # partition's chunk segment [x-cols || x_out-cols] is one contiguous
    # DRAM run -> one large DMA descriptor per partition per chunk.
    xc8 = nc.dram_tensor("xc8", [2 * P * FREE], FP8, kind="ExternalInput")
    wid = nc.dram_tensor("wid", [P, 2, P], FP8, kind="ExternalInput")
    sm = nc.dram_tensor("smalls", [BL, SM_W], F32, kind="ExternalInput")
    out = nc.dram_tensor("out", [P, NCOL], F32, kind="ExternalOutput")

    with tile.TileContext(nc) as tc:
        with (
            tc.tile_pool(name="big", bufs=1) as bp,
            tc.tile_pool(name="acc", bufs=1) as cp,
            tc.tile_pool(name="small", bufs=1) as sp,
            tc.tile_pool(name="work", bufs=1) as wp,
            tc.tile_pool(name="psum", bufs=1, space="PSUM") as pp,
        ):
            # identity weights FIRST on the Sync ring (every matmul gates
            # on them); smalls on the Scalar ring drain concurrently.
            wid_t = sp.tile([P, 2, P], FP8)
            nc.sync.dma_start(out=wid_t[:], in_=wid.ap())
            sm_t = sp.tile([BL, SM_W], F32)
            nc.scalar.dma_start(out=sm_t[:], in_=sm.ap())

            xts = []
            for i, ch in enumerate(CHUNKS):
                xt = bp.tile([P, 2, ch], FP8, tag=f"xt{i}")
                base = 2 * P * CUM[i]
                src = xc8.ap()[base : base + 2 * P * ch]
                src = src.rearrange("(p h n) -> p h n", p=P, h=2)
                nc.sync.dma_start(out=xt[:], in_=src)
                xts.append(xt)

            # ---- early setup (hides in the preamble window) ----
            ones_t = cp.tile([P, 1], F32)
            nc.vector.memset(ones_t[:], 1.0)
            accR = cp.tile([P, NCOL], F32)
            nc.vector.memset(accR[:], 0.0)
            # warm up the ACT table so the load is off the critical path
            warm = cp.tile([1, 1], F32)
            nc.vector.memset(warm[:], 0.0)
            nc.scalar.activation(warm[:], warm[:], ACTF.Abs)

            # psum tiles: 2x ACT spans + 2x DVE spans (2 banks each)
            # = exactly 8 banks.
            pa = [pp.tile([P, 1024], F32, name=f"pa{i}") for i in range(2)]
            pd = [pp.tile([P, 1024], F32, name=f"pd{i}") for i in range(2)]
            # ACT output dummies (bf16)
            ad = [bp.tile([P, 1024], BF16, name=f"ad{i}") for i in range(2)]

            # ---- KLD on the 32-sample rows, vectorized over dims ----
            mu3 = sm_t[:, SM_MU3 : SM_MU3 + 30]
            pos3 = sm_t[:, SM_POS3 : SM_POS3 + 30]
            iotacat = sm_t[:, SM_IOTACAT : SM_IOTACAT + 70]
            iota40 = sm_t[:, SM_IOTACAT + 30 : SM_IOTACAT + 70]
            ybcat = sm_t[:, SM_YBCAT : SM_YBCAT + 70]
            ycat = sm_t[:, SM_YCAT : SM_YCAT + ND + NL]
            mul = sm_t[:, SM_MUL : SM_MUL + NL]

            sel7 = wp.tile([BL, ND + NL], F32)

            ohcat = wp.tile([BL, 70], F32)
            nc.vector.tensor_tensor(ohcat[:], iotacat, ybcat, ALU.is_equal)
            oh = ohcat[:, 0:30]
            oh4 = ohcat[:, 30:70]
            eqcat = wp.tile([BL, ND + NL], F32)
            nc.vector.tensor_tensor(eqcat[:], ycat, ycat, ALU.is_equal)
            eqd = eqcat[:, 0:ND]
            eql = eqcat[:, ND:]

            dist = wp.tile([BL, 30], F32)
            nc.vector.tensor_sub(dist[:], mu3, pos3)
            nc.vector.tensor_mul(dist[:], dist[:], dist[:])
            nc.vector.tensor_mul(oh, oh, dist[:])
            unl = wp.tile([BL, ND], F32)
            nc.vector.tensor_reduce(
                unl[:], dist[:].rearrange("p (d k) -> p d k", k=NPOS),
                AXIS.X, ALU.min,
            )
            lab = wp.tile([BL, ND], F32)
            nc.vector.tensor_reduce(
                lab[:], oh.rearrange("p (d k) -> p d k", k=NPOS),
                AXIS.X, ALU.add,
            )
            nc.vector.tensor_sub(lab[:], lab[:], unl[:])
            nc.vector.tensor_mul(lab[:], lab[:], eqd)
            nc.vector.tensor_add(sel7[:, 0:ND], lab[:], unl[:])

            nc.vector.tensor_mul(oh4, oh4, iota40)
            ysafe = wp.tile([BL, NL], F32)
            nc.vector.tensor_reduce(
                ysafe[:], oh4.rearrange("p (d k) -> p d k", k=NPOS),
                AXIS.X, ALU.add,
            )
            labl = wp.tile([BL, NL], F32)
            nc.vector.tensor_sub(labl[:], mul, ysafe[:])
            nc.vector.tensor_mul(labl[:], labl[:], labl[:])
            nm = wp.tile([BL, NL], F32)
            nc.vector.tensor_scalar(nm[:], mul, -1.0, None, ALU.mult)
            nc.vector.tensor_max(nm[:], mul, nm[:])
            nc.vector.tensor_scalar(nm[:], nm[:], -10.0, 0.0, ALU.add, ALU.max)
            nc.vector.tensor_mul(nm[:], nm[:], nm[:])
            nc.vector.tensor_sub(labl[:], labl[:], nm[:])
            nc.vector.tensor_mul(labl[:], labl[:], eql)
            nc.vector.tensor_add(sel7[:, ND:], labl[:], nm[:])

            nc.vector.tensor_reduce(
                accR[0:BL, COL_KLD : COL_KLD + 1], sel7[:], AXIS.X, ALU.add
            )

            # ---- main loop: PE DoubleRow identity-sub into psum spans,
            # ACT/DVE abs+sum consumers ----
            col_act = 0
            col_dve = N_ACT
            ia = 0
            idv = 0

            for i, spans in enumerate(PLAN):
                xt = xts[i]
                off = 0
                for eng, width in spans:
                    if eng == "act":
                        pt = pa[ia % 2]
                        ia += 1
                    else:
                        pt = pd[idv % 2]
                        idv += 1
                    for j in range(0, width, 512):
                        w = min(512, width - j)
                        nc.tensor.matmul(
                            pt[:, j : j + w],
                            wid_t[:],
                            xt[:, :, off + j : off + j + w],
                            start=True, stop=True,
                            perf_mode=PERF.DoubleRow,
                        )
                    if eng == "act":
                        nc.scalar.activation(
                            ad[ia % 2][:, 0:width], pt[:, 0:width], ACTF.Abs,
                            accum_out=accR[:, col_act : col_act + 1],
                        )
                        col_act += 1
                    else:
                        nc.vector.tensor_reduce(
                            accR[:, col_dve : col_dve + 1], pt[:, 0:width],
                            AXIS.X, ALU.add,
                            apply_absolute_value=True,
                        )
                        col_dve += 1
                    off += width

            nc.sync.dma_start(out=out.ap(), in_=accR[:])

    nc.compile()
    return nc


_NC_CACHE = None


def _get_module():
    global _NC_CACHE
    if _NC_CACHE is None:
        _NC_CACHE = build_module()
    return _NC_CACHE


def make_in_maps(x, x_out, y, mu, disc_pos):
    x = np.asarray(x, dtype=np.float32)
    x_out = np.asarray(x_out, dtype=np.float32)
    y = np.asarray(y, dtype=np.float32)
    mu = np.asarray(mu, dtype=np.float32)
    disc_pos = np.asarray(disc_pos, dtype=np.float32)

    ident = np.eye(P, dtype=np.float32)
    wid = np.stack([ident, -ident], axis=1).astype(NP_FP8)

    iota = np.arange(NPOS, dtype=np.float32)
    in_maps = []
    for i in range(N_CORES):
        s = slice(i * BL, (i + 1) * BL)
        xv = x[s].reshape(P, FREE).astype(NP_FP8)
        yv = x_out[s].reshape(P, FREE).astype(NP_FP8)
        xc8 = np.empty(2 * P * FREE, dtype=NP_FP8)
        p8 = 0
        for k, ch in enumerate(CHUNKS):
            blk = np.stack(
                [xv[:, CUM[k]:CUM[k + 1]], yv[:, CUM[k]:CUM[k + 1]]],
                axis=1,
            )
            n = 2 * P * ch
            xc8[p8:p8 + n] = blk.reshape(-1)
            p8 += n

        mu_s, y_s = mu[s], y[s]
        sm = np.empty((BL, SM_W), dtype=np.float32)
        sm[:, SM_MU3:SM_MU3 + 30] = np.repeat(mu_s[:, :ND], NPOS, axis=1)
        sm[:, SM_POS3:SM_POS3 + 30] = np.tile(disc_pos, ND)
        sm[:, SM_IOTACAT:SM_IOTACAT + 70] = np.tile(iota, ND + NL)
        sm[:, SM_YBCAT:SM_YBCAT + 70] = np.repeat(y_s, NPOS, axis=1)
        sm[:, SM_YCAT:SM_YCAT + ND + NL] = y_s
        sm[:, SM_MUL:SM_MUL + NL] = mu_s[:, ND:ND + NL]

        in_maps.append({"xc8": xc8, "wid": wid, "smalls": sm})
    return in_maps


def combine_partials(partials):
    """partials: [8, P, NCOL] per-core accR -> full (3,) output."""
    p = np.asarray(partials, dtype=np.float64).reshape(N_CORES, P, NCOL)
    s = p.sum(axis=(0, 1)) / B
    recon = s[:COL_KLD].sum()
    kld = s[COL_KLD]
    return np.array([recon, kld, recon + kld], dtype=np.float32)


def run_spmd(x, x_out, y, mu, disc_pos, trace=False, **kw):
    from concourse.bass_utils import run_bass_kernel_spmd

    nc = _get_module()
    in_maps = make_in_maps(x, x_out, y, mu, disc_pos)
    r = run_bass_kernel_spmd(nc, in_maps, list(range(N_CORES)), trace=trace, **kw)
    partials = [r.results[i]["out"] for i in range(N_CORES)]
    return combine_partials(partials), r


def kernel(x, x_out, y, mu, disc_pos):
    out, _ = run_spmd(x, x_out, y, mu, disc_pos)
    return out


if __name__ == "__main__":
    nc = build_module()
    print("module built ok")
